# revision 43
# baseline (speedup 1.0000x reference)
"""Trainium2 Bass kernel for nn_ModelSimplest_11596411699489.

Model: 4D conv (valid, 13^4 kernel, 1->3 ch, 18^4 -> 6^4) + bias + relu
       -> flatten (3888) -> dense (3888->2) + bias -> softmax.  B=512.

Mapping: conv lowered to fp8 DoubleRow matmuls over (z,w)-plane Toeplitz
blocks.  For each output position (ox,oy) and kernel-plane offset
(kx,ky), the contribution of input plane (ox+kx, oy+ky) to the 108
outputs (co,oz,ow) is a [324 x 108] Toeplitz matrix.  Plane rows are
chunked 324 = 3*108 partitions; the (kx,c,ky) tiles are flattened to
507 k-tiles of 108 rows and consumed two-at-a-time by fp8e4
MatmulPerfMode.DoubleRow matmuls (2 k-tiles contracted per streamed
column -> 2x MAC rate vs fp16; measured 162.5 ns per N=384 matmul).
A "hex" access pattern feeds all 6 oy blocks of an ox-row in one N=384
matmul: rhs [108, 2(ktile), 6(oy), 64(b)] with the oy dim striding the
y-planes (the same weight serves block oy at plane y=ky+oy).  All 6
ox-rows accumulate in parallel (6 PSUM banks), 254 DoubleRow matmuls
each, scheduled as a wavefront over d = ox+kx so DMA needs only one
new x row (~370 KB) per wave and each accumulator finishes at a
different wave -- its relu evac (scalar engine) and dense partials
(vector engine scalar_tensor_tensor) overlap the remaining conv.
Filler matmuls after wave 0 keep the PE p-state ramping through the
head DMA window.  The logits are finished by two ones-vector
partition-reduce matmuls plus three fp32 matmuls for the last row's
tail, then softmax on vector/scalar.

Sharding (8 cores): pure data parallel, 64 samples per core.  Each core
computes all 36 (ox,oy) blocks for its batch slice, then the dense
layer + softmax locally -- no collective.  Host concatenates the 8
[64, 2] outputs.

Quantization: x*16 and conv_w*64 in fp8 e4m3fn (descaled by 2^-10 in
the relu+bias activation), fp32 accumulation in PSUM.  Dense + softmax
in fp32.  End-to-end rel err 1.35e-2 (gate 2e-2).  HW exec ~272 us vs
675 us for the fp16 pair/single baseline (2.48x).
"""

import sys

if "/opt/trn_rl_repo" not in sys.path:
    sys.path.insert(0, "/opt/trn_rl_repo")

import numpy as np
import ml_dtypes

E4 = ml_dtypes.float8_e4m3fn

B, S, KS, SO, COUT = 512, 18, 13, 6, 3
NBC = B // 8             # 64 samples per core
P3 = 108                 # partition rows per plane chunk; 3*108 = 324
M = COUT * SO * SO       # 108 outputs per (ox,oy) block
NT = KS * 3 * KS         # 507 k-tiles (kx, c, ky)
NPAIR = (NT + 1) // 2    # 254 DoubleRow matmuls per ox-row

XSCALE, WSCALE = 16.0, 64.0
DESCALE = 1.0 / (XSCALE * WSCALE)

# x SBUF tile free-dim strides (elements): [X(18), c(3), y(18), b(64)]
XSTR = 3 * 18 * 64       # 3456 per X
CSTR = 18 * 64           # 1152 per c
YSTR = 64

_cache = {}


def _tile_order():
    """k-tile t -> (kx, c, ky), flattened kx-major for DMA-friendly order."""
    return [(kx, c, ky) for kx in range(KS) for c in range(3) for ky in range(KS)]


def _build_nc():
    import concourse.mybir as mybir
    import concourse.tile as tile
    from concourse import bacc
    from concourse.ap import AP

    f8 = mybir.dt.float8e4
    f16 = mybir.dt.float16
    f32 = mybir.dt.float32

    nc = bacc.Bacc(num_devices=8)

    xp_d = nc.dram_tensor("xp", [18, 3, P3, 18, NBC], f8, kind="ExternalInput")
    wt_d = nc.dram_tensor("wt", [P3, 2 * NPAIR, 128], f8, kind="ExternalInput")
    cb_d = nc.dram_tensor("cb", [128, 1], f32, kind="ExternalInput")
    wd_d = nc.dram_tensor("wd", [128, 36, 2], f32, kind="ExternalInput")
    db_d = nc.dram_tensor("db", [2, NBC], f32, kind="ExternalInput")
    out_d = nc.dram_tensor("out", [NBC, 2], f32, kind="ExternalOutput")

    tiles = _tile_order()

    with tile.TileContext(nc) as tc:
        with (
            tc.tile_pool(name="xp", bufs=1) as xpool,
            tc.tile_pool(name="wp", bufs=1) as wpool,
            tc.tile_pool(name="fp", bufs=1) as fpool,
            tc.tile_pool(name="sp", bufs=1) as spool,
            tc.tile_pool(name="pp", bufs=1, space="PSUM") as ppool,
            tc.tile_pool(name="dp", bufs=1, space="DRAM") as dpool,
        ):
            xt = xpool.tile([P3, 18, 3, 18, NBC], f8, tag="x")
            wt = wpool.tile([P3, 2 * NPAIR, 128], f8, tag="w")

            # --- DMA schedule, wavefront-need order, 3 queues.  Queues are
            # picked greedily by accumulated bytes so arrival tracks the
            # emission (need) order instead of piling onto one queue. ---
            qs = [nc.sync, nc.scalar, nc.gpsimd]
            qi = [0]

            def dma(out, in_):
                # round-robin: consecutive need-order items always land on
                # different queues, so their transfers overlap
                qs[qi[0] % 3].dma_start(out=out, in_=in_)
                qi[0] += 1

            # head: first weight pair + first x slices, then wave order.
            # wave d uses x row X=d and (for group ox=0) weight chunk kx=d.
            # Early rows go in y-halves so arrival granularity stays fine.
            dma(wt[:, 0:2, :], wt_d[:, 0:2, :])
            dma(xt[:, 0, 0, 0:8, :], xp_d[0, 0, :, 0:8, :])
            dma(wt[:, 2:13, :], wt_d[:, 2:13, :])
            dma(xt[:, 0, 0, 8:18, :], xp_d[0, 0, :, 8:18, :])
            for c in (1, 2):
                dma(xt[:, 0, c, 0:9, :], xp_d[0, c, :, 0:9, :])
                if c == 1:
                    dma(wt[:, 13:26, :], wt_d[:, 13:26, :])
                else:
                    dma(wt[:, 26:39, :], wt_d[:, 26:39, :])
                dma(xt[:, 0, c, 9:18, :], xp_d[0, c, :, 9:18, :])
            for d in range(1, 18):
                for c in range(3):
                    if d <= 3:
                        dma(xt[:, d, c, 0:9, :], xp_d[d, c, :, 0:9, :])
                        dma(xt[:, d, c, 9:18, :], xp_d[d, c, :, 9:18, :])
                    else:
                        dma(xt[:, d, c, :, :], xp_d[d, c])
                if d < KS:
                    # weight chunk kx=d, split in 3 for queue parallelism
                    t0 = d * 39
                    t1 = (d + 1) * 39 + (1 if d == KS - 1 else 0)
                    dma(wt[:, t0 : t0 + 13, :], wt_d[:, t0 : t0 + 13, :])
                    dma(wt[:, t0 + 13 : t0 + 26, :], wt_d[:, t0 + 13 : t0 + 26, :])
                    dma(wt[:, t0 + 26 : t1, :], wt_d[:, t0 + 26 : t1, :])

            # small constants (needed only at evac/dense time)
            cb_t = spool.tile([128, 1], f32, tag="cb")
            nc.scalar.dma_start(out=cb_t[:], in_=cb_d[:])
            wd_t = spool.tile([128, 36, 2], f32, tag="wd")
            nc.scalar.dma_start(out=wd_t[:], in_=wd_d[:])
            ones_t = spool.tile([128, 1], f32, tag="ones")
            nc.gpsimd.memset(ones_t[:], 1.0)

            accs = [
                ppool.tile([128, 512], f32, tag=f"acc{ox}", name=f"acc{ox}")
                for ox in range(6)
            ]
            feats = fpool.tile([128, 6, 6, NBC], f32, tag="feats")
            dacc = ppool.tile([NBC, 2], f32, tag="dacc")
            # dense accumulators, one per class, summed on the Vector engine
            dv = [
                spool.tile([128, NBC], f32, tag=f"dv{c}", name=f"dv{c}")
                for c in range(2)
            ]
            # zero the dense accumulators, then drop dense_b into row 108
            # (feats/wd rows >= 108 are zero) so the ones-reduce matmul
            # adds the bias for free
            nc.vector.memset(dv[0][:], 0.0)
            nc.vector.memset(dv[1][:], 0.0)
            nc.scalar.dma_start(out=dv[0][108:109, :], in_=db_d[0:1, :])
            nc.scalar.dma_start(out=dv[1][108:109, :], in_=db_d[1:2, :])

            xfull = xt[:]
            pstride = xfull.ap[0][0]
            xtensor = xfull.tensor

            def xoff(X, c, ky):
                return X * XSTR + c * CSTR + ky * YSTR

            # --- conv, wavefront order: wave d = all (ox, pair) with
            # ox + kx(pair's later tile) == d.  Each acc[ox] spans waves
            # ox..ox+12; its evac + dense overlap later waves. ---
            waves = [[] for _ in range(18)]
            for ox in range(6):
                for p in range(NPAIR):
                    kxb = tiles[min(2 * p + 1, NT - 1)][0]
                    waves[ox + kxb].append((p, ox))
            for w in waves:
                w.sort()

            def conv_mm(p, ox):
                kx0, c0, ky0 = tiles[2 * p]
                if 2 * p + 1 < NT:
                    kx1, c1, ky1 = tiles[2 * p + 1]
                    o0 = xoff(ox + kx0, c0, ky0)
                    kstride = xoff(ox + kx1, c1, ky1) - o0
                else:
                    o0 = xoff(ox + kx0, c0, ky0)
                    kstride = 0
                rhs = AP(
                    xtensor, o0,
                    [[pstride, P3], [kstride, 2], [YSTR, 6], [1, NBC]],
                )
                nc.tensor.matmul(
                    accs[ox][:, 0 : 6 * NBC],
                    lhsT=wt[:, 2 * p : 2 * p + 2, :],
                    rhs=rhs,
                    start=(p == 0),
                    stop=(p == NPAIR - 1),
                    perf_mode=mybir.MatmulPerfMode.DoubleRow,
                )

            def evac(ox, split=False):
                if split:
                    # per-oy slices so the dense matmuls pipeline with it
                    for oy in range(6):
                        nc.scalar.activation(
                            feats[:, ox, oy, :],
                            accs[ox][:, oy * NBC : (oy + 1) * NBC],
                            mybir.ActivationFunctionType.Relu,
                            bias=cb_t[:],
                            scale=DESCALE,
                        )
                    return
                nc.scalar.activation(
                    feats[:, ox, :, :],
                    accs[ox][:, 0 : 6 * NBC],
                    mybir.ActivationFunctionType.Relu,
                    bias=cb_t[:],
                    scale=DESCALE,
                )

            def dense(ox, oys=range(6)):
                # partial logits on the (otherwise idle) Vector engine:
                # dv[cls] += feats[:, ox, oy, :] * wd[:, blk, cls]
                for oy in oys:
                    blk = ox * 6 + oy
                    for cls in range(2):
                        nc.vector.scalar_tensor_tensor(
                            dv[cls][:],
                            feats[:, ox, oy, :],
                            wd_t[:, blk, cls : cls + 1],
                            dv[cls][:],
                            mybir.AluOpType.mult,
                            mybir.AluOpType.add,
                        )

            warm = ppool.tile([128, NBC], f32, tag="warm")
            dscratch = dpool.tile([128, 12], f32, tag="dscratch")
            for d in range(18):
                for p, ox in waves[d]:
                    conv_mm(p, ox)
                    if p == NPAIR - 1:
                        # relu-evac + vector-dense run under later conv
                        # matmuls; the final ox has none left, so split it
                        # and give half the dense work to the tensor engine
                        evac(ox, split=(ox == 5))
                        dense(ox, oys=range(3) if ox == 5 else range(6))
                        # keep the sync DMA queue warm so the final output
                        # DMA doesn't pay cold-queue completion latency
                        nc.sync.dma_start(
                            out=dscratch[:, 2 * ox : 2 * ox + 2],
                            in_=feats[0:128, ox, 0, 0:2],
                        )
                if d == 0:
                    # fillers bridge DMA-arrival jitter at the head; they
                    # cost nothing when the stream is DMA-gated anyway
                    for i in range(10):
                        rhs = AP(xtensor, 0, [[pstride, P3], [0, 2], [1, NBC]])
                        nc.tensor.matmul(
                            warm[:, :],
                            lhsT=wt[:, 0:2, :],
                            rhs=rhs,
                            start=True,
                            stop=True,
                            perf_mode=mybir.MatmulPerfMode.DoubleRow,
                        )

            # ox=5's last 3 blocks straight on the tensor engine (fp32),
            # then the partition-reduce of the vector accumulators, all one
            # PSUM accumulation group on dacc
            for oy in (3, 4, 5):
                nc.tensor.matmul(
                    dacc[:, :],
                    lhsT=feats[:, 5, oy, :],
                    rhs=wd_t[:, 5 * 6 + oy, :],
                    start=(oy == 3),
                    stop=False,
                    skip_group_check=True,
                )
            for cls in range(2):
                nc.tensor.matmul(
                    dacc[:, cls : cls + 1],
                    lhsT=dv[cls][:],
                    rhs=ones_t[:, 0:1],
                    start=False,
                    stop=(cls == 1),
                    skip_group_check=True,
                )

            # --- softmax over the 2 classes (bias already in dacc), write
            # out.  Exp's accum_out gives the denominator in the same op.
            ex = spool.tile([NBC, 2], f32, tag="ex")
            sm = spool.tile([NBC, 1], f32, tag="sm")
            nc.scalar.activation(
                ex[:], dacc[:], mybir.ActivationFunctionType.Exp,
                accum_out=sm[:],
            )
            rc = spool.tile([NBC, 1], f32, tag="rc")
            nc.vector.reciprocal(rc[:], sm[:])
            pr = spool.tile([NBC, 2], f32, tag="pr")
            nc.vector.tensor_scalar_mul(pr[:], ex[:], rc[:])
            nc.sync.dma_start(out=out_d[:], in_=pr[:], single_packet=True)

    nc.finalize()
    return nc


def _build_wt(conv_w):
    """conv_w [3,1,13,13,13,13] -> [108, 508, 128] fp8 k-tile stack."""
    p = np.arange(P3)
    m = np.arange(M)
    co = m // (SO * SO)
    oz = (m % (SO * SO)) // SO
    ow = m % SO
    cw = (conv_w[:, 0] * WSCALE).astype(np.float32)  # [3,13,13,13,13]

    wt = np.zeros((P3, 2 * NPAIR, 128), np.float32)
    for t, (kx, c, ky) in enumerate(_tile_order()):
        pg = c * P3 + p                       # plane row id in [0,324)
        z = pg // S
        w_ = pg % S
        dz = z[:, None] - oz[None, :]         # [108,108]
        dw = w_[:, None] - ow[None, :]
        valid = (dz >= 0) & (dz < KS) & (dw >= 0) & (dw < KS)
        vals = cw[
            np.broadcast_to(co[None, :], dz.shape),
            kx, ky,
            np.clip(dz, 0, KS - 1),
            np.clip(dw, 0, KS - 1),
        ]
        wt[:, t, :M] = np.where(valid, vals, 0.0)
    return wt.astype(E4)


def _build_inputs(x, conv_w, conv_b, dense_w, dense_b):
    wt = _build_wt(conv_w)

    m = np.arange(M)
    co = m // (SO * SO)
    oz = (m % (SO * SO)) // SO
    ow = m % SO

    cb = np.zeros((128, 1), np.float32)
    cb[:M, 0] = conv_b[co]

    # dense weights regrouped per (ox,oy) block: feat = co*6^4 + ox*6^3 +
    # oy*6^2 + oz*6 + ow
    wd = np.zeros((128, 36, 2), np.float32)
    for blk in range(36):
        ox, oy = blk // 6, blk % 6
        f = co * SO**4 + ox * SO**3 + oy * SO**2 + oz * SO + ow
        wd[:M, blk, :] = dense_w[:, f].T

    db = np.tile(dense_b[:, None].astype(np.float32), (1, NBC))

    in_maps = []
    for core in range(8):
        xs = x[NBC * core : NBC * (core + 1), 0]      # [64, X, Y, z, w]
        t = xs.transpose(1, 3, 4, 2, 0)               # [X, z, w, Y, b]
        t = t.reshape(S, 3, P3, S, NBC)               # [X, c, p, y, b]
        xq = np.ascontiguousarray(t * XSCALE).astype(E4)
        in_maps.append({"xp": xq, "wt": wt, "cb": cb, "wd": wd, "db": db})
    return in_maps


def _run(in_maps, trace=False):
    from concourse.bass_utils import run_bass_kernel_spmd

    if "nc" not in _cache:
        _cache["nc"] = _build_nc()
    try:
        return run_bass_kernel_spmd(
            _cache["nc"], in_maps, list(range(8)), trace=trace
        )
    except Exception:
        # transient NRT device errors have been observed; retry once
        return run_bass_kernel_spmd(
            _cache["nc"], in_maps, list(range(8)), trace=trace
        )


def kernel(x, conv_w, conv_b, dense_w, dense_b, _trace=False):
    x = np.asarray(x, np.float32)
    conv_w = np.asarray(conv_w, np.float32)
    conv_b = np.asarray(conv_b, np.float32)
    dense_w = np.asarray(dense_w, np.float32)
    dense_b = np.asarray(dense_b, np.float32)

    in_maps = _build_inputs(x, conv_w, conv_b, dense_w, dense_b)
    res = _run(in_maps, trace=_trace)
    out = np.concatenate([res.results[i]["out"] for i in range(8)], axis=0)
    if _trace:
        return out, res
    return out


# revision 47
# speedup vs baseline: 1.0121x; 1.0121x over previous
"""Trainium2 Bass kernel for nn_ModelSimplest_11596411699489.

Model: 4D conv (valid, 13^4 kernel, 1->3 ch, 18^4 -> 6^4) + bias + relu
       -> flatten (3888) -> dense (3888->2) + bias -> softmax.  B=512.

Mapping: conv lowered to fp8 DoubleRow matmuls over (z,w)-plane Toeplitz
blocks.  For each output position (ox,oy) and kernel-plane offset
(kx,ky), the contribution of input plane (ox+kx, oy+ky) to the 108
outputs (co,oz,ow) is a [324 x 108] Toeplitz matrix.  Plane rows are
chunked 324 = 3*108 partitions; the (kx,c,ky) tiles are flattened to
507 k-tiles of 108 rows and consumed two-at-a-time by fp8e4
MatmulPerfMode.DoubleRow matmuls (2 k-tiles contracted per streamed
column -> 2x MAC rate vs fp16; measured 162.5 ns per N=384 matmul).
A "hex" access pattern feeds all 6 oy blocks of an ox-row in one N=384
matmul: rhs [108, 2(ktile), 6(oy), 64(b)] with the oy dim striding the
y-planes (the same weight serves block oy at plane y=ky+oy).  All 6
ox-rows accumulate in parallel (6 PSUM banks), 254 DoubleRow matmuls
each, scheduled as a wavefront over d = ox+kx so DMA needs only one
new x row (~370 KB) per wave and each accumulator finishes at a
different wave -- its relu evac (scalar engine) and dense partials
(vector engine scalar_tensor_tensor) overlap the remaining conv.
Filler matmuls after wave 0 keep the PE p-state ramping through the
head DMA window.  The logits are finished by two ones-vector
partition-reduce matmuls plus three fp32 matmuls for the last row's
tail, then softmax on vector/scalar.

Sharding (8 cores): pure data parallel, 64 samples per core.  Each core
computes all 36 (ox,oy) blocks for its batch slice, then the dense
layer + softmax locally -- no collective.  Host concatenates the 8
[64, 2] outputs.

Quantization: x*16 and conv_w*64 in fp8 e4m3fn (descaled by 2^-10 in
the relu+bias activation), fp32 accumulation in PSUM.  Dense + softmax
in fp32.  End-to-end rel err 1.35e-2 (gate 2e-2).  HW exec ~272 us vs
675 us for the fp16 pair/single baseline (2.48x).
"""

import sys

if "/opt/trn_rl_repo" not in sys.path:
    sys.path.insert(0, "/opt/trn_rl_repo")

import numpy as np
import ml_dtypes

E4 = ml_dtypes.float8_e4m3fn

B, S, KS, SO, COUT = 512, 18, 13, 6, 3
NBC = B // 8             # 64 samples per core
P3 = 108                 # partition rows per plane chunk; 3*108 = 324
M = COUT * SO * SO       # 108 outputs per (ox,oy) block
NT = KS * 3 * KS         # 507 k-tiles (kx, c, ky)
NPAIR = (NT + 1) // 2    # 254 DoubleRow matmuls per ox-row

XSCALE, WSCALE = 16.0, 64.0
DESCALE = 1.0 / (XSCALE * WSCALE)

# x SBUF tile free-dim strides (elements): [X(18), c(3), y(18), b(64)]
XSTR = 3 * 18 * 64       # 3456 per X
CSTR = 18 * 64           # 1152 per c
YSTR = 64

_cache = {}


def _tile_order():
    """k-tile t -> (kx, c, ky), flattened kx-major for DMA-friendly order."""
    return [(kx, c, ky) for kx in range(KS) for c in range(3) for ky in range(KS)]


def _build_nc():
    import concourse.mybir as mybir
    import concourse.tile as tile
    from concourse import bacc
    from concourse.ap import AP

    f8 = mybir.dt.float8e4
    f16 = mybir.dt.float16
    f32 = mybir.dt.float32

    nc = bacc.Bacc(num_devices=8)

    xp_d = nc.dram_tensor("xp", [18, 3, P3, 18, NBC], f8, kind="ExternalInput")
    wt_d = nc.dram_tensor("wt", [P3, 2 * NPAIR, 128], f8, kind="ExternalInput")
    cb_d = nc.dram_tensor("cb", [128, 1], f32, kind="ExternalInput")
    wd_d = nc.dram_tensor("wd", [128, 36, 2], f32, kind="ExternalInput")
    db_d = nc.dram_tensor("db", [2, NBC], f32, kind="ExternalInput")
    out_d = nc.dram_tensor("out", [NBC, 2], f32, kind="ExternalOutput")

    tiles = _tile_order()

    with tile.TileContext(nc) as tc:
        with (
            tc.tile_pool(name="xp", bufs=1) as xpool,
            tc.tile_pool(name="wp", bufs=1) as wpool,
            tc.tile_pool(name="fp", bufs=1) as fpool,
            tc.tile_pool(name="sp", bufs=1) as spool,
            tc.tile_pool(name="pp", bufs=1, space="PSUM") as ppool,
        ):
            xt = xpool.tile([P3, 18, 3, 18, NBC], f8, tag="x")
            wt = wpool.tile([P3, 2 * NPAIR, 128], f8, tag="w")

            # --- DMA schedule, wavefront-need order, 3 queues.  Queues are
            # picked greedily by accumulated bytes so arrival tracks the
            # emission (need) order instead of piling onto one queue. ---
            qs = [nc.sync, nc.scalar, nc.gpsimd]
            qi = [0]

            def dma(out, in_):
                # round-robin: consecutive need-order items always land on
                # different queues, so their transfers overlap
                qs[qi[0] % 3].dma_start(out=out, in_=in_)
                qi[0] += 1

            # head: first weight pair + first x slices, then wave order.
            # wave d uses x row X=d and (for group ox=0) weight chunk kx=d.
            # Early rows go in y-halves so arrival granularity stays fine.
            dma(wt[:, 0:2, :], wt_d[:, 0:2, :])
            dma(xt[:, 0, 0, 0:8, :], xp_d[0, 0, :, 0:8, :])
            dma(wt[:, 2:13, :], wt_d[:, 2:13, :])
            dma(xt[:, 0, 0, 8:18, :], xp_d[0, 0, :, 8:18, :])
            for c in (1, 2):
                dma(xt[:, 0, c, 0:9, :], xp_d[0, c, :, 0:9, :])
                if c == 1:
                    dma(wt[:, 13:26, :], wt_d[:, 13:26, :])
                else:
                    dma(wt[:, 26:39, :], wt_d[:, 26:39, :])
                dma(xt[:, 0, c, 9:18, :], xp_d[0, c, :, 9:18, :])
            for d in range(1, 18):
                for c in range(3):
                    if d <= 3:
                        dma(xt[:, d, c, 0:9, :], xp_d[d, c, :, 0:9, :])
                        dma(xt[:, d, c, 9:18, :], xp_d[d, c, :, 9:18, :])
                    else:
                        dma(xt[:, d, c, :, :], xp_d[d, c])
                if d < KS:
                    # weight chunk kx=d, split in 3 for queue parallelism
                    t0 = d * 39
                    t1 = (d + 1) * 39 + (1 if d == KS - 1 else 0)
                    dma(wt[:, t0 : t0 + 13, :], wt_d[:, t0 : t0 + 13, :])
                    dma(wt[:, t0 + 13 : t0 + 26, :], wt_d[:, t0 + 13 : t0 + 26, :])
                    dma(wt[:, t0 + 26 : t1, :], wt_d[:, t0 + 26 : t1, :])

            # small constants (needed only at evac/dense time)
            cb_t = spool.tile([128, 1], f32, tag="cb")
            nc.scalar.dma_start(out=cb_t[:], in_=cb_d[:])
            wd_t = spool.tile([128, 36, 2], f32, tag="wd")
            nc.scalar.dma_start(out=wd_t[:], in_=wd_d[:])
            ones_t = spool.tile([128, 1], f32, tag="ones")
            nc.gpsimd.memset(ones_t[:], 1.0)

            accs = [
                ppool.tile([128, 512], f32, tag=f"acc{ox}", name=f"acc{ox}")
                for ox in range(6)
            ]
            feats = fpool.tile([128, 6, 6, NBC], f32, tag="feats")
            dacc = ppool.tile([NBC, 2], f32, tag="dacc")
            # dense accumulators, one per class, summed on the Vector engine
            dv = [
                spool.tile([128, NBC], f32, tag=f"dv{c}", name=f"dv{c}")
                for c in range(2)
            ]
            # zero the dense accumulators, then drop dense_b into row 108
            # (feats/wd rows >= 108 are zero) so the ones-reduce matmul
            # adds the bias for free
            nc.vector.memset(dv[0][:], 0.0)
            nc.vector.memset(dv[1][:], 0.0)
            nc.scalar.dma_start(out=dv[0][108:109, :], in_=db_d[0:1, :])
            nc.scalar.dma_start(out=dv[1][108:109, :], in_=db_d[1:2, :])

            xfull = xt[:]
            pstride = xfull.ap[0][0]
            xtensor = xfull.tensor

            def xoff(X, c, ky):
                return X * XSTR + c * CSTR + ky * YSTR

            # --- conv, wavefront order: wave d = all (ox, pair) with
            # ox + kx(pair's later tile) == d.  Each acc[ox] spans waves
            # ox..ox+12; its evac + dense overlap later waves. ---
            waves = [[] for _ in range(18)]
            for ox in range(6):
                for p in range(NPAIR):
                    kxb = tiles[min(2 * p + 1, NT - 1)][0]
                    waves[ox + kxb].append((p, ox))
            for w in waves:
                w.sort()

            def conv_mm(p, ox):
                kx0, c0, ky0 = tiles[2 * p]
                if 2 * p + 1 < NT:
                    kx1, c1, ky1 = tiles[2 * p + 1]
                    o0 = xoff(ox + kx0, c0, ky0)
                    kstride = xoff(ox + kx1, c1, ky1) - o0
                else:
                    o0 = xoff(ox + kx0, c0, ky0)
                    kstride = 0
                rhs = AP(
                    xtensor, o0,
                    [[pstride, P3], [kstride, 2], [YSTR, 6], [1, NBC]],
                )
                nc.tensor.matmul(
                    accs[ox][:, 0 : 6 * NBC],
                    lhsT=wt[:, 2 * p : 2 * p + 2, :],
                    rhs=rhs,
                    start=(p == 0),
                    stop=(p == NPAIR - 1),
                    perf_mode=mybir.MatmulPerfMode.DoubleRow,
                )

            def evac(ox, split=False):
                if split:
                    # per-oy slices so the dense matmuls pipeline with it
                    for oy in range(6):
                        nc.scalar.activation(
                            feats[:, ox, oy, :],
                            accs[ox][:, oy * NBC : (oy + 1) * NBC],
                            mybir.ActivationFunctionType.Relu,
                            bias=cb_t[:],
                            scale=DESCALE,
                        )
                    return
                nc.scalar.activation(
                    feats[:, ox, :, :],
                    accs[ox][:, 0 : 6 * NBC],
                    mybir.ActivationFunctionType.Relu,
                    bias=cb_t[:],
                    scale=DESCALE,
                )

            def dense(ox, oys=range(6)):
                # partial logits on the (otherwise idle) Vector engine:
                # dv[cls] += feats[:, ox, oy, :] * wd[:, blk, cls]
                for oy in oys:
                    blk = ox * 6 + oy
                    for cls in range(2):
                        nc.vector.scalar_tensor_tensor(
                            dv[cls][:],
                            feats[:, ox, oy, :],
                            wd_t[:, blk, cls : cls + 1],
                            dv[cls][:],
                            mybir.AluOpType.mult,
                            mybir.AluOpType.add,
                        )

            warm = ppool.tile([128, NBC], f32, tag="warm")
            for d in range(18):
                for p, ox in waves[d]:
                    conv_mm(p, ox)
                    if p == NPAIR - 1:
                        # relu-evac + vector-dense run under later conv
                        # matmuls; the final ox has none left, so split it
                        # and give half the dense work to the tensor engine
                        evac(ox, split=(ox == 5))
                        dense(ox, oys=range(3) if ox == 5 else range(6))
                if d == 0:
                    # fillers bridge DMA-arrival jitter at the head; they
                    # cost nothing when the stream is DMA-gated anyway
                    for i in range(10):
                        rhs = AP(xtensor, 0, [[pstride, P3], [0, 2], [1, NBC]])
                        nc.tensor.matmul(
                            warm[:, :],
                            lhsT=wt[:, 0:2, :],
                            rhs=rhs,
                            start=True,
                            stop=True,
                            perf_mode=mybir.MatmulPerfMode.DoubleRow,
                        )

            # ox=5's last 3 blocks straight on the tensor engine (fp32),
            # then the partition-reduce of the vector accumulators, all one
            # PSUM accumulation group on dacc
            for oy in (3, 4, 5):
                nc.tensor.matmul(
                    dacc[:, :],
                    lhsT=feats[:, 5, oy, :],
                    rhs=wd_t[:, 5 * 6 + oy, :],
                    start=(oy == 3),
                    stop=False,
                    skip_group_check=True,
                )
            for cls in range(2):
                nc.tensor.matmul(
                    dacc[:, cls : cls + 1],
                    lhsT=dv[cls][:],
                    rhs=ones_t[:, 0:1],
                    start=False,
                    stop=(cls == 1),
                    skip_group_check=True,
                )

            # --- softmax over the 2 classes (bias already in dacc) ---
            ex = spool.tile([NBC, 2], f32, tag="ex")
            nc.scalar.activation(ex[:], dacc[:], mybir.ActivationFunctionType.Exp)
            sm = spool.tile([NBC, 1], f32, tag="sm")
            nc.vector.reduce_sum(sm[:], ex[:], axis=mybir.AxisListType.X)
            rc = spool.tile([NBC, 1], f32, tag="rc")
            nc.vector.reciprocal(rc[:], sm[:])
            pr = spool.tile([NBC, 2], f32, tag="pr")
            nc.vector.tensor_scalar_mul(pr[:], ex[:], rc[:])
            nc.sync.dma_start(out=out_d[:], in_=pr[:])

    nc.finalize()
    return nc


def _build_wt(conv_w):
    """conv_w [3,1,13,13,13,13] -> [108, 508, 128] fp8 k-tile stack."""
    p = np.arange(P3)
    m = np.arange(M)
    co = m // (SO * SO)
    oz = (m % (SO * SO)) // SO
    ow = m % SO
    cw = (conv_w[:, 0] * WSCALE).astype(np.float32)  # [3,13,13,13,13]

    wt = np.zeros((P3, 2 * NPAIR, 128), np.float32)
    for t, (kx, c, ky) in enumerate(_tile_order()):
        pg = c * P3 + p                       # plane row id in [0,324)
        z = pg // S
        w_ = pg % S
        dz = z[:, None] - oz[None, :]         # [108,108]
        dw = w_[:, None] - ow[None, :]
        valid = (dz >= 0) & (dz < KS) & (dw >= 0) & (dw < KS)
        vals = cw[
            np.broadcast_to(co[None, :], dz.shape),
            kx, ky,
            np.clip(dz, 0, KS - 1),
            np.clip(dw, 0, KS - 1),
        ]
        wt[:, t, :M] = np.where(valid, vals, 0.0)
    return wt.astype(E4)


def _build_inputs(x, conv_w, conv_b, dense_w, dense_b):
    wt = _build_wt(conv_w)

    m = np.arange(M)
    co = m // (SO * SO)
    oz = (m % (SO * SO)) // SO
    ow = m % SO

    cb = np.zeros((128, 1), np.float32)
    cb[:M, 0] = conv_b[co]

    # dense weights regrouped per (ox,oy) block: feat = co*6^4 + ox*6^3 +
    # oy*6^2 + oz*6 + ow
    wd = np.zeros((128, 36, 2), np.float32)
    for blk in range(36):
        ox, oy = blk // 6, blk % 6
        f = co * SO**4 + ox * SO**3 + oy * SO**2 + oz * SO + ow
        wd[:M, blk, :] = dense_w[:, f].T

    db = np.tile(dense_b[:, None].astype(np.float32), (1, NBC))

    in_maps = []
    for core in range(8):
        xs = x[NBC * core : NBC * (core + 1), 0]      # [64, X, Y, z, w]
        t = xs.transpose(1, 3, 4, 2, 0)               # [X, z, w, Y, b]
        t = t.reshape(S, 3, P3, S, NBC)               # [X, c, p, y, b]
        xq = np.ascontiguousarray(t * XSCALE).astype(E4)
        in_maps.append({"xp": xq, "wt": wt, "cb": cb, "wd": wd, "db": db})
    return in_maps


def _run(in_maps, trace=False):
    from concourse.bass_utils import run_bass_kernel_spmd

    if "nc" not in _cache:
        _cache["nc"] = _build_nc()
    try:
        return run_bass_kernel_spmd(
            _cache["nc"], in_maps, list(range(8)), trace=trace
        )
    except Exception:
        # transient NRT device errors have been observed; retry once
        return run_bass_kernel_spmd(
            _cache["nc"], in_maps, list(range(8)), trace=trace
        )


def kernel(x, conv_w, conv_b, dense_w, dense_b, _trace=False):
    x = np.asarray(x, np.float32)
    conv_w = np.asarray(conv_w, np.float32)
    conv_b = np.asarray(conv_b, np.float32)
    dense_w = np.asarray(dense_w, np.float32)
    dense_b = np.asarray(dense_b, np.float32)

    in_maps = _build_inputs(x, conv_w, conv_b, dense_w, dense_b)
    res = _run(in_maps, trace=_trace)
    out = np.concatenate([res.results[i]["out"] for i in range(8)], axis=0)
    if _trace:
        return out, res
    return out
